# revision 1
# baseline (speedup 1.0000x reference)
"""DiscriminativeLoss segment-reduce kernel for 8x TRN2 NeuronCores.

Data-parallel over batch: core i processes image i (16,512,512) + mask.
Device computes per-image: segment sums/counts (33,17) and varsum (1,33)
(= sum over pixels of each segment of relu(||x-mu_seg||-0.5)^2).
Host finishes the tiny (33,16) math: means, dist/reg losses, reduction.

Pixel layout on chip: partition p owns pixels [Apos*p, Apos*(p+1)),
position a within partition; pixel n = Apos*p + a.
"""

from contextlib import ExitStack

import numpy as np

import concourse.bass as bass
import concourse.tile as tile
import concourse.mybir as mybir
from concourse import bass_utils

F32 = mybir.dt.float32
BF16 = mybir.dt.bfloat16
I32 = mybir.dt.int32
U8 = mybir.dt.uint8

B = 8          # batch (one image per core)
E = 16         # embedding channels
EC = E + 1     # + ones column
K = 33         # segments (0 = background)
P = 128        # partitions
DELTA_V = 0.5
DELTA_D = 1.5
ALPHA, BETA, GAMMA = 1.0, 1.0, 0.001

N_FULL = 512 * 512


def geom(n_pix):
    a = n_pix // P          # positions per partition
    mh = a // 2             # chunk-pairs for pass 2
    slab_m = min(32, mh)    # m's per slab
    achunk = min(128, a)    # positions per pass-1 chunk group
    return dict(N=n_pix, A=a, MH=mh, SLAB_M=slab_m, N_SLAB=mh // slab_m,
                PSB=min(16, slab_m), ACHUNK=achunk, NCH=a // achunk,
                LCHUNK=achunk)


def _bcast(ap_in, count):
    """Append a step-0 broadcast dim of `count` to an AP."""
    return bass.AP(tensor=ap_in.tensor, offset=ap_in.offset,
                   ap=list(ap_in.ap) + [[0, count]])


def build_kernel(tc: tile.TileContext, embs: list, ids8: bass.AP,
                 ids8t: bass.AP, out_s: bass.AP, out_v: bass.AP, g):
    import os as _os
    SKIP = set(_os.environ.get("KSKIP", "").split(","))
    nc = tc.nc
    N, A, MH, SLAB_M, N_SLAB, PSB, ACHUNK, NCH = (
        g["N"], g["A"], g["MH"], g["SLAB_M"], g["N_SLAB"], g["PSB"],
        g["ACHUNK"], g["NCH"])
    LCHUNK = g["LCHUNK"]

    with ExitStack() as ctx:
        singles = ctx.enter_context(tc.tile_pool(name="singles", bufs=1))
        stage = ctx.enter_context(tc.tile_pool(name="stage", bufs=2))
        onehot_pool = ctx.enter_context(tc.tile_pool(name="onehot", bufs=3))
        psum = ctx.enter_context(tc.tile_pool(name="psum", bufs=1, space="PSUM"))
        psum_z = ctx.enter_context(tc.tile_pool(name="psum_z", bufs=3, space="PSUM"))
        dram = ctx.enter_context(tc.tile_pool(name="dram", bufs=1, space="DRAM"))
        p2 = ctx.enter_context(tc.tile_pool(name="p2", bufs=2))

        # ---------------- persistent SBUF tensors ----------------
        xbf = singles.tile([P, A, EC], BF16)       # [p, a, e|ones]
        ids_bf = singles.tile([P, A], BF16)
        ssq = singles.tile([P, A], F32)
        t_px = singles.tile([P, A], BF16)
        iota_rep = singles.tile([P, ACHUNK, K], BF16)
        kcol = singles.tile([P, 1], F32)           # j' mod 64
        wtab = singles.tile([P, EC], BF16)         # [-2mu|msq] rows 0:33, 64:97
        s_sb = singles.tile([K, EC], F32)
        recip = singles.tile([K, 1], F32)
        mu32 = singles.tile([K, EC], F32)
        stage_s = singles.tile([K, EC], F32)
        stage_v4 = singles.tile([P, K], F32)
        nc.vector.memset(stage_v4, 0.0)

        ids8_v = ids8.rearrange("(p a) -> p a", p=P)

        # ---------------- ids load (per chunk, u8 -> bf16) ----------------
        IDC = max(A // 8, 1)
        IDH = IDC // 2
        for ci in range(A // IDC):
            a0 = ci * IDC
            # padded 3D tile defeats the DIRECT2D lowering (1-wait limit)
            idsu = stage.tile([P, 2, IDH + 8], U8, tag="idsu")
            nc.sync.dma_start(out=idsu[:, :, 0:IDH],
                              in_=ids8_v[:, a0:a0 + IDC].rearrange(
                                  "p (h m) -> p h m", h=2))
            nc.vector.tensor_copy(
                out=ids_bf[:, a0:a0 + IDC].rearrange("p (h m) -> p h m", h=2),
                in_=idsu[:, :, 0:IDH])

        # iota 0..K-1 repeated ACHUNK times, as bf16
        iota_i = singles.tile([P, ACHUNK, K], I32)
        nc.gpsimd.iota(iota_i, pattern=[[0, ACHUNK], [1, K]], base=0,
                       channel_multiplier=0)
        nc.vector.tensor_copy(out=iota_rep, in_=iota_i)

        # kcol[j'] = j' mod 64
        kcol_i = singles.tile([P, 1], I32)
        nc.gpsimd.iota(kcol_i, pattern=[[0, 1]], base=0, channel_multiplier=1)
        kmod = singles.tile([P, 1], I32)
        nc.vector.tensor_scalar(out=kmod, in0=kcol_i, scalar1=63, scalar2=None,
                                op0=mybir.AluOpType.bitwise_and)
        nc.vector.tensor_copy(out=kcol, in_=kmod)

        nc.vector.memset(xbf[:, :, E], 1.0)
        negd = singles.tile([P, 1], F32)
        nc.vector.memset(negd, -DELTA_V)

        # ---------------- X load + bf16 cast + ssq ----------------
        EH = E // len(embs)
        emb_vs = [e.rearrange("e (p a) -> p e a", p=P) for e in embs]
        for ci in range(A // LCHUNK):
            a0 = ci * LCHUNK
            xs = stage.tile([P, E, LCHUNK], F32, tag="xstage")
            for gi, ev in enumerate(emb_vs):
                nc.sync.dma_start(out=xs[:, gi * EH:(gi + 1) * EH, :],
                                  in_=ev[:, :, a0:a0 + LCHUNK])
            nc.scalar.copy(out=xbf[:, a0:a0 + LCHUNK, 0:E],
                           in_=xs.rearrange("p e a -> p a e"))
            prod = stage.tile([P, LCHUNK, E], BF16, tag="prodstage")
            xv0 = xbf[:, a0:a0 + LCHUNK, 0:E]
            nc.gpsimd.tensor_tensor(out=prod, in0=xv0, in1=xv0,
                                    op=mybir.AluOpType.mult)
            nc.vector.tensor_reduce(out=ssq[:, a0:a0 + LCHUNK], in_=prod,
                                    axis=mybir.AxisListType.X,
                                    op=mybir.AluOpType.add)

        # ---------------- pass 1: segment sums -> psum (K, EC) ----------------
        ps_s = psum.tile([K, EC], F32)
        if "p1" in SKIP:
            nc.vector.memset(ps_s, 1.0)
        for ci in ([], range(NCH))["p1" not in SKIP]:
            a0 = ci * ACHUNK
            oh = onehot_pool.tile([P, ACHUNK, K], BF16, tag="oh1")
            ids_bc = _bcast(ids_bf[:, a0:a0 + ACHUNK], K)
            nc.vector.tensor_tensor(out=oh, in0=iota_rep, in1=ids_bc,
                                    op=mybir.AluOpType.is_equal)
            for j in range(ACHUNK):
                a = a0 + j
                nc.tensor.matmul(ps_s, lhsT=oh[:, j, :], rhs=xbf[:, a, :],
                                 start=(a == 0), stop=(a == A - 1))

        # means etc. (tiny, K partitions)
        nc.vector.tensor_copy(out=s_sb, in_=ps_s)
        cnt_c = singles.tile([K, 1], F32)
        nc.vector.tensor_scalar_max(cnt_c, s_sb[:, E:E + 1], 1.0)
        nc.vector.reciprocal(recip, cnt_c)
        nc.vector.tensor_scalar_mul(mu32, s_sb, recip)
        musq = singles.tile([K, E], F32)
        nc.vector.tensor_tensor(out=musq, in0=mu32[:, 0:E], in1=mu32[:, 0:E],
                                op=mybir.AluOpType.mult)
        nc.vector.tensor_reduce(out=mu32[:, E:E + 1], in_=musq,
                                axis=mybir.AxisListType.X,
                                op=mybir.AluOpType.add)
        nc.vector.memset(wtab, 0.0)
        wneg = singles.tile([K, EC], F32)
        nc.vector.tensor_scalar_mul(wneg, mu32, -2.0)
        nc.vector.tensor_copy(out=wneg[:, E:E + 1], in_=mu32[:, E:E + 1])
        nc.vector.tensor_copy(out=wtab[0:K, :], in_=wneg)
        nc.vector.tensor_copy(out=wtab[64:64 + K, :], in_=wneg)

        nc.vector.tensor_copy(out=stage_s, in_=ps_s)
        nc.gpsimd.dma_start(out=out_s, in_=stage_s)

        # ---------------- pass 2: gather + d2 chain -> t ----------------
        if "p2" in SKIP:
            nc.vector.memset(t_px, 0.25)
        GS = 1
        GSM = GS * SLAB_M
        zsl = None
        for s in ([], range(N_SLAB))["p2" not in SKIP]:
            m0 = s * SLAB_M
            # rep[j', m, j] = ids8t[(m0+m + MH*(j'>=64))*P + j]
            rep = p2.tile([P, SLAB_M, P], U8, tag="rep")
            for h in range(2):
                src = bass.AP(tensor=ids8t.tensor,
                              offset=ids8t.offset + (m0 + MH * h) * P,
                              ap=[[0, 64], [1, SLAB_M * P]])
                nc.gpsimd.dma_start(
                    out=rep[64 * h:64 * (h + 1), :, :].rearrange(
                        "r m j -> r (m j)"),
                    in_=src)
            oht = p2.tile([P, SLAB_M, P], BF16, tag="oht")
            if "p2oht" in SKIP:
                nc.vector.memset(oht, 0.0)
            else:
                nc.vector.tensor_scalar(out=oht, in0=rep, scalar1=kcol,
                                        scalar2=None,
                                        op0=mybir.AluOpType.is_equal)
            if s % GS == 0:
                zfull = p2.tile([P, 2, GSM, EC], BF16, tag="zsl")
            zsl = zfull[:, :, (s % GS) * SLAB_M:(s % GS + 1) * SLAB_M, :]
            if "p2mm" in SKIP:
                nc.vector.memset(zfull, 0.0)
            for b in ([], range(SLAB_M // PSB))["p2mm" not in SKIP]:
                pzA = psum_z.tile([P, PSB * EC], F32, tag="pzA")
                pzB = psum_z.tile([P, PSB * EC], F32, tag="pzB")
                for mi in range(PSB):
                    m = b * PSB + mi
                    nc.tensor.matmul(pzA[:, mi * EC:(mi + 1) * EC],
                                     lhsT=oht[0:64, m, :], rhs=wtab[0:64, :],
                                     start=True, stop=True,
                                     tile_position=(0, 0))
                    nc.tensor.matmul(pzB[:, mi * EC:(mi + 1) * EC],
                                     lhsT=oht[64:128, m, :],
                                     rhs=wtab[64:128, :],
                                     start=True, stop=True,
                                     tile_position=(64, 0))
                nc.scalar.copy(out=zsl[:, 0, b * PSB:(b + 1) * PSB, :],
                               in_=pzA.rearrange("p (m c) -> p m c", c=EC))
                nc.scalar.copy(out=zsl[:, 1, b * PSB:(b + 1) * PSB, :],
                               in_=pzB.rearrange("p (m c) -> p m c", c=EC))
            if s % GS != GS - 1:
                continue
            g0 = (s // GS) * GSM
            for h in range(2):
                a0 = h * MH + g0
                xv = xbf[:, a0:a0 + GSM, :]
                prod2 = p2.tile([P, GSM, EC], BF16, tag="prod2")
                nc.vector.tensor_tensor(out=prod2, in0=xv,
                                        in1=zfull[:, h, :, :],
                                        op=mybir.AluOpType.mult)
                d2 = p2.tile([P, GSM], F32, tag="d2")
                nc.vector.tensor_reduce(out=d2, in_=prod2,
                                        axis=mybir.AxisListType.X,
                                        op=mybir.AluOpType.add)
                nc.vector.scalar_tensor_tensor(out=d2, in0=d2, scalar=1.0,
                                               in1=ssq[:, a0:a0 + GSM],
                                               op0=mybir.AluOpType.mult,
                                               op1=mybir.AluOpType.add)
                nc.vector.tensor_scalar_max(d2, d2, 0.0)
                dd = p2.tile([P, GSM], F32, tag="dd")
                nc.scalar.sqrt(dd, d2)
                nc.scalar.activation(out=dd, in_=dd,
                                     func=mybir.ActivationFunctionType.Relu,
                                     bias=negd, scale=1.0)
                nc.scalar.activation(out=t_px[:, a0:a0 + GSM], in_=dd,
                                     func=mybir.ActivationFunctionType.Square)

        # ---------------- pass 3: varsum = segsum(t) -> psum (1, K) --------
        ps_v = psum.tile([1, K], F32)
        if "p3" in SKIP:
            nc.vector.memset(ps_v, 1.0)
        for ci in ([], range(NCH))["p3" not in SKIP]:
            a0 = ci * ACHUNK
            oh = onehot_pool.tile([P, ACHUNK, K], BF16, tag="oh1")
            ids_bc = _bcast(ids_bf[:, a0:a0 + ACHUNK], K)
            nc.vector.tensor_tensor(out=oh, in0=iota_rep, in1=ids_bc,
                                    op=mybir.AluOpType.is_equal)
            for j in range(ACHUNK):
                a = a0 + j
                nc.tensor.matmul(ps_v, lhsT=t_px[:, a:a + 1], rhs=oh[:, j, :],
                                 start=(a == 0), stop=(a == A - 1))
        nc.vector.tensor_copy(out=stage_v4[0:1, :], in_=ps_v)
        nc.gpsimd.dma_start(out=out_v, in_=stage_v4[0:4, :])


def _split_excess_waits(nc, keep=1):
    """walrus can't encode >1 sem-wait on queue/engine instruction structs;
    move excess waits to standalone EventSemaphore instructions (sound:
    tile semaphores are monotonic within a kernel)."""
    f = nc.m.functions[0]
    for blk in f.blocks:
        newlist = []
        changed = False
        for ins in blk.instructions:
            si = ins.sync_info
            waits = list(si.on_wait) if si is not None else []
            if len(waits) > keep:
                for wi, w in enumerate(waits[:-keep]):
                    ev = mybir.InstEventSemaphore(
                        name=f"{ins.name}_w{wi}", ins=[], outs=[])
                    ev.engine = ins.engine
                    ev.sync_info = mybir.SyncInfo(on_wait=[w], on_update=[])
                    newlist.append(ev)
                ins.sync_info = mybir.SyncInfo(on_wait=waits[-keep:],
                                               on_update=list(si.on_update))
                changed = True
            newlist.append(ins)
        if changed:
            blk.instructions = newlist


_CACHE = {}


def _get_nc(n_pix=N_FULL):
    key = ("nc", n_pix)
    if key in _CACHE:
        return _CACHE[key]
    g = geom(n_pix)
    nc = bass.Bass("TRN2", num_devices=B)
    nsplit = 2 if n_pix >= 512 * 512 else 1
    embs = [nc.dram_tensor(f"emb{i}", [E // nsplit, n_pix], F32,
                           kind="ExternalInput").ap() for i in range(nsplit)]
    ids8 = nc.dram_tensor("ids8", [n_pix], U8, kind="ExternalInput").ap()
    ids8t = nc.dram_tensor("ids8t", [n_pix], U8, kind="ExternalInput").ap()
    out_s = nc.dram_tensor("out_s", [K, EC], F32, kind="ExternalOutput").ap()
    out_v = nc.dram_tensor("out_v", [4, K], F32, kind="ExternalOutput").ap()
    with tile.TileContext(nc) as tc:
        build_kernel(tc, embs, ids8, ids8t, out_s, out_v, g)
    nc._n_emb_split = nsplit
    _split_excess_waits(nc)
    _CACHE[key] = nc
    return nc


def _finish_host(s_arr, v_arr):
    sums = s_arr[:, 0:E].astype(np.float64)
    counts = s_arr[:, E].astype(np.float64)
    varsum = v_arr.astype(np.float64)
    counts_c = np.maximum(counts, 1.0)
    means = sums / counts_c[:, None]
    present = counts[1:] > 0
    n_inst = float(present.sum())
    var_loss = np.sum(np.where(present, varsum[1:] / counts_c[1:], 0.0)) \
        / max(n_inst, 1.0)
    m = means[1:]
    dsq = np.sum((m[:, None, :] - m[None, :, :]) ** 2, axis=-1)
    dmat = np.sqrt(np.maximum(dsq, 0.0))
    pair_mask = (np.triu(np.ones((K - 1, K - 1), bool), 1)
                 & present[:, None] & present[None, :])
    n_pairs = float(pair_mask.sum())
    dist_term = np.maximum(2.0 * DELTA_D - dmat, 0.0) ** 2
    dist_loss = np.sum(np.where(pair_mask, dist_term, 0.0)) / max(n_pairs, 1.0)
    dist_loss = dist_loss * float(n_inst > 1.0)
    mean_norms = np.sqrt(np.sum(m * m, axis=1))
    reg_loss = np.sum(np.where(present, mean_norms, 0.0)) / max(n_inst, 1.0)
    valid = float(n_inst > 0.0)
    return var_loss * valid, dist_loss * valid, reg_loss * valid, valid


def kernel(embeddings: np.ndarray, instance_masks: np.ndarray) -> np.ndarray:
    embeddings = np.ascontiguousarray(embeddings, dtype=np.float32)
    instance_masks = np.ascontiguousarray(instance_masks, dtype=np.int32)
    n_pix = embeddings.shape[2] * embeddings.shape[3]
    nc = _get_nc(n_pix)
    nsplit = getattr(nc, "_n_emb_split", 2)
    eh = E // nsplit
    in_maps = []
    for i in range(B):
        u8 = instance_masks[i].reshape(n_pix).astype(np.uint8)
        u8t = np.ascontiguousarray(u8.reshape(P, n_pix // P).T).reshape(n_pix)
        m = {"ids8": u8, "ids8t": u8t}
        for gi in range(nsplit):
            m[f"emb{gi}"] = embeddings[i].reshape(E, n_pix)[gi * eh:(gi + 1) * eh]
        in_maps.append(m)
    res = bass_utils.run_bass_kernel_spmd(nc, in_maps, core_ids=list(range(B)))
    globals()["LAST_RESULTS"] = res
    vs, ds, rs, valids = [], [], [], []
    for r in res.results:
        v, d, rg, va = _finish_host(r["out_s"], r["out_v"].sum(axis=0))
        vs.append(v); ds.append(d); rs.append(rg); valids.append(va)
    vsum = max(float(np.sum(valids)), 1.0)
    var_loss = float(np.sum(vs)) / vsum
    dist_loss = float(np.sum(ds)) / vsum
    reg_loss = float(np.sum(rs)) / vsum
    total = ALPHA * var_loss + BETA * dist_loss + GAMMA * reg_loss
    return np.array([total, var_loss, dist_loss, reg_loss], dtype=np.float32)



# revision 47
# speedup vs baseline: 3.2425x; 3.2425x over previous
"""DiscriminativeLoss segment-reduce kernel for 8x TRN2 NeuronCores (v4).

Data-parallel over batch: core i processes image i. Per-core plan:
  pass1: segment sums+counts (K, EC) via per-k is_equal onehot (DVE 4x mode)
         + per-column matmuls into PSUM. The full onehot [P, K, A] persists
         in SBUF and is reused by pass3 (no rebuild).
  means: tiny chain on 33 partitions; mu broadcast to all 128 partitions
         (transposed) via a DRAM bounce.
  pass2: per-pixel d^2 = sum_e (x_e - mu_id,e)^2 computed in a transposed
         layout xt[(g,e), c]: GPSIMD indirect_copy gathers mu per pixel
         (per-core wrapped indices), DVE does (x-mu) and square, PE reduces
         over the 16 e-partitions with a block-ones matmul, Act exits PSUM,
         DMA relayouts d^2 back to pixel-major.
  t-chain: t = relu(sqrt(d^2) - 0.5)^2 on Act.
  pass3: varsum[k] = segsum(t) via the persisted onehot (matmuls only).
Host: bf16 pre-conversion + layouts (numpy), final tiny loss math.
"""

from contextlib import ExitStack

import numpy as np
import ml_dtypes

import concourse.bass as bass
import concourse.tile as tile
import concourse.mybir as mybir
from concourse import bass_utils

F32 = mybir.dt.float32
BF16 = mybir.dt.bfloat16
U16 = mybir.dt.uint16
FP8 = mybir.dt.float8e4

B = 8          # batch (one image per core)
E = 16         # embedding channels
EC = E + 1     # + ones column
K = 33         # segments (0 = background)
P = 128        # partitions
G = 8          # pixel groups (16 partitions each)
DELTA_V = 0.5
DELTA_D = 1.5
ALPHA, BETA, GAMMA = 1.0, 1.0, 0.001

N_FULL = 512 * 512


def build_kernel(tc: tile.TileContext, xpix_d, xt_d, idsb_d, idsw_d,
                 ones8_d, i33_d, out_s, out_v, n_pix):
    nc = tc.nc
    A = n_pix // P           # positions per partition (2048)
    C = n_pix // G           # columns per group (32768)
    ACH = 512                # pass1 x-DMA a-chunk
    OCH = 512                # onehot build chunk
    DCH = 2048               # pass2 c-chunk (= 128 a-columns)
    ICH = 1024               # indirect_copy max num_valid
    AD = DCH // 16           # a-columns per pass2 chunk (128)

    with ExitStack() as ctx:
        sing = ctx.enter_context(tc.tile_pool(name="sing", bufs=1))
        psum = ctx.enter_context(tc.tile_pool(name="psum", bufs=1, space="PSUM"))
        psd = ctx.enter_context(tc.tile_pool(name="psd", bufs=3, space="PSUM"))
        dram = ctx.enter_context(tc.tile_pool(name="dram", bufs=1, space="DRAM"))

        # ---------------- persistent SBUF ----------------
        oh = sing.tile([P, K, A], BF16)      # full onehot, built in p1, reused p3
        idsb = sing.tile([P, A], BF16)
        idsw = sing.tile([P, A], U16)
        t_px = sing.tile([P, A], BF16)
        dpm = sing.tile([P, A], BF16)        # d = sqrt(d2), pixel-major
        mu128 = sing.tile([P, K], BF16)
        ones8 = sing.tile([P, G], BF16)

        i33 = sing.tile([K, K], BF16)
        nc.sync.dma_start(out=idsb, in_=idsb_d)
        nc.sync.dma_start(out=idsw, in_=idsw_d)
        nc.sync.dma_start(out=ones8, in_=ones8_d)
        nc.sync.dma_start(out=i33, in_=i33_d)

        # ---------------- pass 1: segment sums -> psum (K, EC) ----------
        # onehot chunks taper (big first, small last) so the matmul stream
        # starts early and the tail drains fast
        ps_s = psum.tile([K, EC], F32)
        ochunks = []
        rem = 0
        for w in (OCH * 2, OCH, OCH // 2, OCH // 2):
            ochunks.append((rem, w))
            rem += w
        assert rem == A
        with tc.tile_pool(name="xch", bufs=2) as xch:
            for a0, w in ochunks:
                for k in range(K):
                    nc.vector.tensor_scalar(out=oh[:, k, a0:a0 + w],
                                            in0=idsb[:, a0:a0 + w],
                                            scalar1=float(k), scalar2=None,
                                            op0=mybir.AluOpType.is_equal)
            for ci in range(A // ACH):
                a0 = ci * ACH
                xc = xch.tile([P, EC, ACH], FP8, tag="xc")
                nc.sync.dma_start(out=xc, in_=xpix_d[:, :, a0:a0 + ACH])
                for j in range(ACH):
                    a = a0 + j
                    nc.tensor.matmul(ps_s, lhsT=oh[:, :, a], rhs=xc[:, :, j],
                                     start=(a == 0),
                                     stop=(a == n_pix // P - 1))

        # ---------------- means (tiny, K partitions) ---------------------
        s_sb = sing.tile([K, EC], F32)
        nc.vector.tensor_copy(out=s_sb, in_=ps_s)
        nc.sync.dma_start(out=out_s, in_=s_sb)
        cnt = sing.tile([K, 1], F32)
        nc.vector.tensor_scalar_max(cnt, s_sb[:, E:E + 1], 1.0)
        recip = sing.tile([K, 1], F32)
        nc.vector.reciprocal(recip, cnt)
        mu_bf = sing.tile([K, E], BF16)
        nc.vector.tensor_scalar_mul(mu_bf, s_sb[:, 0:E], recip)
        # mu128[16g+e, k] = mu[k, e]: replicate mu columns 8x on DVE, then
        # transpose+broadcast through PE with an identity rhs.
        murep = sing.tile([K, P], BF16)
        mu_ap = mu_bf[:, :]
        src = bass.AP(tensor=mu_ap.tensor, offset=mu_ap.offset,
                      ap=list(mu_ap.ap[:-1]) + [[0, G], [1, E]])
        nc.vector.tensor_copy(out=murep.rearrange("k (g e) -> k g e", g=G),
                              in_=src)
        pmu = psum.tile([P, K], F32)
        nc.tensor.matmul(pmu, lhsT=murep, rhs=i33, start=True, stop=True,
                         skip_group_check=True)
        nc.vector.tensor_copy(out=mu128, in_=pmu)

        # ---------------- pass 2 + pipelined t-chain + pass 3 ------------
        # column map: c = DCH*m + 128*jo + ji  <->  pixel (p=16g+jo, a=AD*m+ji)
        dwork = ctx.enter_context(tc.tile_pool(name="dwork", bufs=4))
        dwk2 = ctx.enter_context(tc.tile_pool(name="dwk2", bufs=2))
        dwk3 = ctx.enter_context(tc.tile_pool(name="dwk3", bufs=3))
        ps_v = psum.tile([K, 1], F32)
        # DRAM scratch for d in full-image [g][jo][a] layout
        dscr = dram.tile([G, 16 * A], BF16)
        scr_ap = dscr[:, :]
        ND = C // DCH
        # read-back batches (chunk index ranges), tapered small at the end
        rb = [(0, 6), (6, 10), (10, 14), (14, 15), (15, 16)]
        for m in range(ND):
            if True:
                c0 = m * DCH
                xtc = dwork.tile([P, DCH], BF16, tag="xtc")
                nc.sync.dma_start(out=xtc, in_=xt_d[:, c0:c0 + DCH])
                muc = dwork.tile([P, DCH], BF16, tag="muc")
                for h in range(DCH // ICH):
                    lo = c0 + h * ICH
                    nc.gpsimd.indirect_copy(
                        out=muc[:, h * ICH:(h + 1) * ICH], data=mu128,
                        idxs=idsw[:, lo // 16:(lo + ICH) // 16],
                        i_know_ap_gather_is_preferred=True)
                # v = x - mu (in place into xtc), v2 = v*v (into muc);
                # square split DVE(1280)/Act(768) to balance engine load
                SQA = 384
                nc.vector.tensor_tensor(out=xtc, in0=xtc, in1=muc,
                                        op=mybir.AluOpType.subtract)
                v2c = muc
                nc.vector.tensor_tensor(out=v2c[:, 0:DCH - SQA],
                                        in0=xtc[:, 0:DCH - SQA],
                                        in1=xtc[:, 0:DCH - SQA],
                                        op=mybir.AluOpType.mult)
                nc.scalar.activation(out=v2c[:, DCH - SQA:DCH],
                                     in_=xtc[:, DCH - SQA:DCH],
                                     func=mybir.ActivationFunctionType.Square)
                # d2[g, c] = sum over the 16 e-partitions of group g
                dsb = dwk3.tile([G, DCH], BF16, tag="dsb")
                for h in range(DCH // ICH):
                    pd = psd.tile([G, ICH], F32, tag="pd")
                    for s in range(ICH // 512):
                        lo = h * ICH + s * 512
                        nc.tensor.matmul(pd[:, s * 512:(s + 1) * 512],
                                         lhsT=ones8, rhs=v2c[:, lo:lo + 512],
                                         start=True, stop=True,
                                         skip_group_check=True)
                    # fused psum exit: d = sqrt(d2), bf16
                    nc.scalar.sqrt(dsb[:, h * ICH:(h + 1) * ICH], pd)
                # write d to DRAM scratch in [g][jo][a] image layout
                dst = bass.AP(tensor=scr_ap.tensor,
                              offset=scr_ap.offset + AD * m,
                              ap=[[16 * A, G], [A, 16], [1, AD]])
                nc.scalar.dma_start(
                    out=dst, in_=dsb.rearrange("g (jo ji) -> g jo ji", jo=16))
            for (mlo, mhi) in rb:
                if m != mhi - 1:
                    continue
                # read back a-range [AD*mlo, AD*mhi) pixel-major, then
                # t = relu(d - dv)^2 and pass3 matmuls for that range
                a0, a1 = AD * mlo, AD * mhi
                src = bass.AP(tensor=scr_ap.tensor, offset=scr_ap.offset + a0,
                              ap=[[A, P], [1, a1 - a0]])
                nc.sync.dma_start(out=dpm[:, a0:a1], in_=src)
                rl = dwk2.tile([P, 6 * AD], BF16, tag="rl")
                nc.vector.tensor_scalar(out=rl[:, 0:a1 - a0], in0=dpm[:, a0:a1],
                                        scalar1=-DELTA_V, scalar2=0.0,
                                        op0=mybir.AluOpType.add,
                                        op1=mybir.AluOpType.max)
                nc.vector.tensor_tensor(out=t_px[:, a0:a1],
                                        in0=rl[:, 0:a1 - a0],
                                        in1=rl[:, 0:a1 - a0],
                                        op=mybir.AluOpType.mult)
                for j in range(a0, a1):
                    nc.tensor.matmul(ps_v, lhsT=oh[:, :, j],
                                     rhs=t_px[:, j:j + 1],
                                     start=(j == 0), stop=(j == A - 1),
                                     skip_group_check=True)

        vst = sing.tile([K, 1], F32)
        nc.vector.tensor_copy(out=vst, in_=ps_v)
        nc.sync.dma_start(out=out_v, in_=vst)


def _split_excess_waits(nc, keep=1):
    """walrus can't encode >1 sem-wait on queue/engine instruction structs;
    move excess waits to standalone EventSemaphore instructions (sound:
    tile semaphores are monotonic within a kernel)."""
    f = nc.m.functions[0]
    for blk in f.blocks:
        newlist = []
        changed = False
        for ins in blk.instructions:
            si = ins.sync_info
            waits = list(si.on_wait) if si is not None else []
            if len(waits) > keep:
                for wi, w in enumerate(waits[:-keep]):
                    ev = mybir.InstEventSemaphore(
                        name=f"{ins.name}_w{wi}", ins=[], outs=[])
                    ev.engine = ins.engine
                    ev.sync_info = mybir.SyncInfo(on_wait=[w], on_update=[])
                    newlist.append(ev)
                ins.sync_info = mybir.SyncInfo(on_wait=waits[-keep:],
                                               on_update=list(si.on_update))
                changed = True
            newlist.append(ins)
        if changed:
            blk.instructions = newlist


_CACHE = {}


def _get_nc(n_pix=N_FULL):
    key = ("nc", n_pix)
    if key in _CACHE:
        return _CACHE[key]
    A = n_pix // P
    nc = bass.Bass("TRN2", num_devices=B)
    xpix_d = nc.dram_tensor("xpix", [P, EC, A], FP8, kind="ExternalInput").ap()
    xt_d = nc.dram_tensor("xt", [P, n_pix // G], BF16, kind="ExternalInput").ap()
    idsb_d = nc.dram_tensor("idsb", [P, A], BF16, kind="ExternalInput").ap()
    idsw_d = nc.dram_tensor("idsw", [P, A], U16, kind="ExternalInput").ap()
    ones8_d = nc.dram_tensor("ones8", [P, G], BF16, kind="ExternalInput").ap()
    i33_d = nc.dram_tensor("i33", [K, K], BF16, kind="ExternalInput").ap()
    out_s = nc.dram_tensor("out_s", [K, EC], F32, kind="ExternalOutput").ap()
    out_v = nc.dram_tensor("out_v", [K, 1], F32, kind="ExternalOutput").ap()
    with tile.TileContext(nc) as tc:
        build_kernel(tc, xpix_d, xt_d, idsb_d, idsw_d, ones8_d, i33_d,
                     out_s, out_v, n_pix)
    _split_excess_waits(nc)
    _CACHE[key] = nc
    return nc


def _finish_host(s_arr, varsum):
    sums = s_arr[:, 0:E].astype(np.float64)
    counts = s_arr[:, E].astype(np.float64)
    varsum = varsum.astype(np.float64)
    counts_c = np.maximum(counts, 1.0)
    means = sums / counts_c[:, None]
    present = counts[1:] > 0
    n_inst = float(present.sum())
    var_loss = np.sum(np.where(present, varsum[1:] / counts_c[1:], 0.0)) \
        / max(n_inst, 1.0)
    m = means[1:]
    dsq = np.sum((m[:, None, :] - m[None, :, :]) ** 2, axis=-1)
    dmat = np.sqrt(np.maximum(dsq, 0.0))
    pair_mask = (np.triu(np.ones((K - 1, K - 1), bool), 1)
                 & present[:, None] & present[None, :])
    n_pairs = float(pair_mask.sum())
    dist_term = np.maximum(2.0 * DELTA_D - dmat, 0.0) ** 2
    dist_loss = np.sum(np.where(pair_mask, dist_term, 0.0)) / max(n_pairs, 1.0)
    dist_loss = dist_loss * float(n_inst > 1.0)
    mean_norms = np.sqrt(np.sum(m * m, axis=1))
    reg_loss = np.sum(np.where(present, mean_norms, 0.0)) / max(n_inst, 1.0)
    valid = float(n_inst > 0.0)
    return var_loss * valid, dist_loss * valid, reg_loss * valid, valid


def kernel(embeddings: np.ndarray, instance_masks: np.ndarray) -> np.ndarray:
    embeddings = np.ascontiguousarray(embeddings, dtype=np.float32)
    instance_masks = np.ascontiguousarray(instance_masks, dtype=np.int32)
    n_pix = embeddings.shape[2] * embeddings.shape[3]
    A = n_pix // P
    C = n_pix // G
    nc = _get_nc(n_pix)
    in_maps = []
    for i in range(B):
        xf = embeddings[i].reshape(E, n_pix)
        xpix = np.empty((P, EC, A), dtype=ml_dtypes.float8_e4m3)
        xpix[:, 0:E, :] = xf.reshape(E, P, A).transpose(1, 0, 2)
        xpix[:, E, :] = 1.0
        # xt[16g+e, c] with c = 2048m + 128jo + ji, pixel (p=16g+jo, a=128m+ji)
        xq = xf.reshape(E, G, 16, 16, 128)          # e g jo m ji
        xt = np.ascontiguousarray(
            xq.transpose(1, 0, 3, 2, 4).reshape(P, C)
        ).astype(ml_dtypes.bfloat16)
        ids = instance_masks[i].reshape(n_pix)
        idsb = ids.reshape(P, A).astype(ml_dtypes.bfloat16)
        # idsw[16g+jl, 128m+8jo+jh] = id(pixel p=16g+jo, a=128m+16jh+jl)
        idq = ids.reshape(G, 16, 16, 8, 16)         # g jo m jh jl
        idsw = np.ascontiguousarray(
            idq.transpose(0, 4, 2, 1, 3).reshape(P, A)
        ).astype(np.uint16)
        ones8 = np.zeros((P, G), dtype=ml_dtypes.bfloat16)
        for g in range(G):
            ones8[16 * g:16 * g + 16, g] = 1.0
        i33 = np.eye(K, dtype=ml_dtypes.bfloat16)
        in_maps.append({"xpix": xpix, "xt": xt, "idsb": idsb, "idsw": idsw,
                        "ones8": ones8, "i33": i33})
    res = bass_utils.run_bass_kernel_spmd(nc, in_maps, core_ids=list(range(B)))
    globals()["LAST_RESULTS"] = res
    vs, ds, rs, valids = [], [], [], []
    for r in res.results:
        v, d, rg, va = _finish_host(r["out_s"], r["out_v"][:, 0])
        vs.append(v); ds.append(d); rs.append(rg); valids.append(va)
    vsum = max(float(np.sum(valids)), 1.0)
    var_loss = float(np.sum(vs)) / vsum
    dist_loss = float(np.sum(ds)) / vsum
    reg_loss = float(np.sum(rs)) / vsum
    total = ALPHA * var_loss + BETA * dist_loss + GAMMA * reg_loss
    return np.array([total, var_loss, dist_loss, reg_loss], dtype=np.float32)
